# revision 6
# baseline (speedup 1.0000x reference)
"""BatchHardTripletLoss Trainium2 kernel (8 NeuronCores, SPMD).

Math: emb = concat(a,p,n) [3B, D]; labels tiled so same-label group of row r
is {a_i, p_i, n_i} with i = r mod B.  dist = sqrt(relu(d2)) is monotone in
d2 = sq_i + sq_j - 2*dot(e_i, e_j), so row max/min commute with sqrt:
  pos_max_d2[r] = max over the 2 partner rows (and 0 for self)
  neg_min_d2[r] = sq_r + min_j!=same (sq_j - 2 dot)
Each core handles 1536 rows x all 12288 cols. PE computes -2*dot into PSUM
(rows' lhsT pre-scaled by -2 on host); a tiny accumulate-matmul adds BIG on
the 3 same-label diagonals (made core-invariant by rotating each core's
column data by its row offset, host-side); one fused DVE tensor_tensor_reduce
per [128,2048] PSUM group adds the column sq and min-accumulates per row.
pos_max via small per-tile fused dot-reduces; regularizer via one abs-sum
reduce. Final sqrt/relu/means on host from tiny [128, 25] per-core outputs.
"""

import os
import sys

import numpy as np

try:
    import ml_dtypes

    BF16 = ml_dtypes.bfloat16
except ImportError:  # pragma: no cover
    BF16 = None

for _p in ("/opt/trn_rl_repo", os.path.expanduser("~/.axon_site/_ro/trn_rl_repo")):
    if os.path.isdir(_p) and _p not in sys.path:
        sys.path.insert(0, _p)
        break

B = 4096
D = 128
NCORES = 8
GW = 2048  # columns per PSUM group (4 banks)
MARGIN = 0.4
ALPHA = 0.01
BIG = float(2**20)

_CACHE = {}


def _build(b, ncores, gw):
    from contextlib import ExitStack

    import concourse.tile as tile
    from concourse import bacc, mybir

    tb = 3 * b
    rpc = tb // ncores  # rows per core
    rt_n = rpc // 128  # row tiles per core
    ng = tb // gw  # column groups
    nmm = gw // 512  # matmuls per group
    f32 = mybir.dt.float32
    bf16 = mybir.dt.bfloat16  # 1 cycle/row on PE, pipelined LDWEIGHTS
    Alu = mybir.AluOpType

    nc = bacc.Bacc("TRN2", target_bir_lowering=False, debug=False, num_devices=ncores)
    rhs_d = nc.dram_tensor("rhs", [128, tb], bf16, kind="ExternalInput")
    sqb_d = nc.dram_tensor("sqb", [128, tb], bf16, kind="ExternalInput")
    lhs_d = nc.dram_tensor("lhs", [128, rpc], bf16, kind="ExternalInput")
    identv_d = nc.dram_tensor("identv", [128, 128], bf16, kind="ExternalInput")
    bigI_d = nc.dram_tensor("bigI", [128, 128], bf16, kind="ExternalInput")
    e_d = nc.dram_tensor("erow", [rpc, 128], f32, kind="ExternalInput")
    o1_d = nc.dram_tensor("o1row", [rpc, 128], f32, kind="ExternalInput")
    o2_d = nc.dram_tensor("o2row", [rpc, 128], f32, kind="ExternalInput")
    s12_d = nc.dram_tensor("s12", [128, rt_n], f32, kind="ExternalInput")
    s13_d = nc.dram_tensor("s13", [128, rt_n], f32, kind="ExternalInput")
    out_d = nc.dram_tensor("out", [128, 2 * rt_n + 1], f32, kind="ExternalOutput")

    with tile.TileContext(nc) as tc, ExitStack() as ctx:
        singles = ctx.enter_context(tc.tile_pool(name="singles", bufs=1))
        psum_pool = ctx.enter_context(tc.tile_pool(name="psum", bufs=2, space="PSUM"))

        rhs_sb = singles.tile([128, tb], bf16)
        sqb_sb = singles.tile([128, tb], bf16)
        lhs_sb = singles.tile([128, rpc], bf16)
        for g in range(ng):
            sl = slice(g * gw, (g + 1) * gw)
            nc.sync.dma_start(out=rhs_sb[:, sl], in_=rhs_d[:, sl])
            nc.sync.dma_start(out=sqb_sb[:, sl], in_=sqb_d[:, sl])
        nc.sync.dma_start(out=lhs_sb[:], in_=lhs_d[:])

        e_sb = singles.tile([128, rt_n, 128], f32)
        o1_sb = singles.tile([128, rt_n, 128], f32)
        o2_sb = singles.tile([128, rt_n, 128], f32)
        nc.sync.dma_start(out=e_sb[:], in_=e_d[:].rearrange("(t p) d -> p t d", p=128))
        nc.sync.dma_start(
            out=o1_sb[:], in_=o1_d[:].rearrange("(t p) d -> p t d", p=128)
        )
        nc.sync.dma_start(
            out=o2_sb[:], in_=o2_d[:].rearrange("(t p) d -> p t d", p=128)
        )
        s12_sb = singles.tile([128, rt_n], f32)
        s13_sb = singles.tile([128, rt_n], f32)
        nc.sync.dma_start(out=s12_sb[:], in_=s12_d[:])
        nc.sync.dma_start(out=s13_sb[:], in_=s13_d[:])

        ident = singles.tile([128, 128], bf16)
        bigI = singles.tile([128, 128], bf16)
        nc.sync.dma_start(out=ident[:], in_=identv_d[:])
        nc.sync.dma_start(out=bigI[:], in_=bigI_d[:])

        negmin = singles.tile([128, rt_n], f32)
        negmin6 = singles.tile([128, rt_n, ng], f32)
        posd1 = singles.tile([128, rt_n], f32)
        posd2 = singles.tile([128, rt_n], f32)
        posd = singles.tile([128, rt_n], f32)
        abss = singles.tile([128, 1], f32)

        rhs_r = rhs_sb[:]
        sqb_r = sqb_sb[:]
        lhs_r = lhs_sb[:]
        ident_r = ident[:]
        bigI_r = bigI[:]

        for rt in range(rt_n):
            lhs_slice = lhs_r[:, rt * 128 : (rt + 1) * 128]
            # same-label diagonals sit at local column t*b + rt*128 (core
            # invariant thanks to the host-side column rotation)
            mask_cols = [t * b + rt * 128 for t in range(3)]
            for g in range(ng):
                ps = psum_pool.tile([128, gw], f32)
                cm = None
                for c in mask_cols:
                    if g * gw <= c < (g + 1) * gw:
                        cm = c - g * gw
                for s in range(nmm):
                    # -2*dot
                    nc.tensor.matmul(
                        ps[:, s * 512 : (s + 1) * 512],
                        lhs_slice,
                        rhs_r[:, g * gw + s * 512 : g * gw + (s + 1) * 512],
                        start=True,
                        stop=False,
                    )
                for s in range(nmm):
                    # += sq[j]  (identity-weighted copy of the sq row-bcast)
                    masked_here = cm is not None and s * 512 <= cm < (s + 1) * 512
                    nc.tensor.matmul(
                        ps[:, s * 512 : (s + 1) * 512],
                        ident_r,
                        sqb_r[:, g * gw + s * 512 : g * gw + (s + 1) * 512],
                        start=False,
                        stop=not masked_here,
                    )
                if cm is not None:
                    # += BIG on the same-label diagonal
                    nc.tensor.matmul(
                        ps[:, cm : cm + 128],
                        ident_r,
                        bigI_r,
                        start=False,
                        stop=True,
                    )
                nc.vector.tensor_reduce(
                    out=negmin6[:, rt, g : g + 1],
                    in_=ps[:],
                    axis=mybir.AxisListType.X,
                    op=Alu.min,
                )
            nc.vector.tensor_reduce(
                out=negmin[:, rt : rt + 1],
                in_=negmin6[:, rt, :],
                axis=mybir.AxisListType.X,
                op=Alu.min,
            )

        # pos path: d2(e,o) = (sq_e + sq_o) - 2*sum_d(e*o), batched over tiles
        prod = singles.tile([128, rt_n, 128], f32)
        dots1 = singles.tile([128, rt_n], f32)
        dots2 = singles.tile([128, rt_n], f32)
        prod2 = singles.tile([128, rt_n, 128], f32)
        nc.gpsimd.tensor_mul(prod[:], e_sb[:], o1_sb[:])
        nc.vector.tensor_reduce(
            out=dots1[:], in_=prod[:], axis=mybir.AxisListType.X, op=Alu.add
        )
        nc.gpsimd.tensor_mul(prod2[:], e_sb[:], o2_sb[:])
        nc.vector.tensor_reduce(
            out=dots2[:], in_=prod2[:], axis=mybir.AxisListType.X, op=Alu.add
        )
        nc.vector.tensor_scalar(
            out=dots1[:], in0=dots1[:], scalar1=-2.0, scalar2=None, op0=Alu.mult
        )
        nc.vector.tensor_scalar(
            out=dots2[:], in0=dots2[:], scalar1=-2.0, scalar2=None, op0=Alu.mult
        )
        nc.vector.tensor_add(out=posd1[:], in0=dots1[:], in1=s12_sb[:])
        nc.vector.tensor_add(out=posd2[:], in0=dots2[:], in1=s13_sb[:])
        nc.vector.tensor_max(out=posd[:], in0=posd1[:], in1=posd2[:])

        nc.vector.tensor_reduce(
            out=abss[:],
            in_=rhs_sb[:, 0:rpc],
            axis=mybir.AxisListType.X,
            op=Alu.add,
            apply_absolute_value=True,
        )

        nc.sync.dma_start(out=out_d[:, 0:rt_n], in_=negmin[:])
        nc.sync.dma_start(out=out_d[:, rt_n : 2 * rt_n], in_=posd[:])
        nc.sync.dma_start(out=out_d[:, 2 * rt_n : 2 * rt_n + 1], in_=abss[:])

    nc.compile()
    return nc


def _host_prepare(a, p, n, b, ncores):
    tb = 3 * b
    rpc = tb // ncores
    rt_n = rpc // 128
    emb = np.concatenate([a, p, n], axis=0).astype(np.float32)  # [3b, D]
    embT = np.ascontiguousarray(emb.T)  # [D, 3b]
    sq = (emb * emb).sum(axis=1, dtype=np.float32)  # [3b]

    in_maps = []
    for k in range(ncores):
        r0 = k * rpc
        rot = (np.arange(tb) + r0) % tb  # local col j holds global col j+r0
        rhs_k = np.ascontiguousarray(embT[:, rot]).astype(BF16)
        sqb_k = np.ascontiguousarray(
            np.broadcast_to(sq[rot][None, :], (128, tb))
        ).astype(BF16)
        lhs_k = np.ascontiguousarray(-2.0 * embT[:, r0 : r0 + rpc]).astype(BF16)
        idx = np.arange(r0, r0 + rpc)
        i = idx % b
        w = idx // b
        o1_idx = ((w + 1) % 3) * b + i
        o2_idx = ((w + 2) % 3) * b + i
        s12_k = np.ascontiguousarray((sq[idx] + sq[o1_idx]).reshape(rt_n, 128).T)
        s13_k = np.ascontiguousarray((sq[idx] + sq[o2_idx]).reshape(rt_n, 128).T)
        in_maps.append(
            {
                "rhs": rhs_k,
                "sqb": sqb_k,
                "lhs": lhs_k,
                "erow": np.ascontiguousarray(emb[idx]),
                "o1row": np.ascontiguousarray(emb[o1_idx]),
                "o2row": np.ascontiguousarray(emb[o2_idx]),
                "s12": s12_k,
                "s13": s13_k,
                "identv": np.eye(128, dtype=np.float32).astype(BF16),
                "bigI": (np.eye(128, dtype=np.float32) * np.float32(BIG)).astype(BF16),
            }
        )
    return in_maps, sq


def _host_finalize(outs, sq, b, ncores):
    """outs: list (per core) of [128, 2*rt_n+1] fp32 device outputs."""
    tb = 3 * b
    rpc = tb // ncores
    rt_n = rpc // 128
    loss_sum = 0.0
    abs_sum = 0.0
    for k in range(ncores):
        o = np.asarray(outs[k], dtype=np.float64)
        idx = np.arange(k * rpc, (k + 1) * rpc)
        sq_r = sq[idx].reshape(rt_n, 128).T  # [p, rt]
        neg_d2 = o[:, 0:rt_n] + sq_r
        pos_d2 = o[:, rt_n : 2 * rt_n]
        neg = np.sqrt(np.maximum(neg_d2, 0.0))
        pos = np.sqrt(np.maximum(pos_d2, 0.0))
        loss_sum += np.maximum(pos - neg + MARGIN, 0.0).sum()
        abs_sum += o[:, 2 * rt_n].sum()
    loss = loss_sum / tb
    sq_sum = sq.sum(dtype=np.float64)
    cnt = tb * D
    reg = (sq_sum - 2.0 * abs_sum + cnt) / cnt
    return np.float32(loss + ALPHA * reg)


def kernel(a, p, n):
    from concourse.bass_utils import run_bass_kernel_spmd

    a = np.asarray(a, dtype=np.float32)
    p = np.asarray(p, dtype=np.float32)
    n = np.asarray(n, dtype=np.float32)
    assert a.shape == (B, D) and p.shape == (B, D) and n.shape == (B, D)

    key = (B, NCORES, GW)
    if key not in _CACHE:
        _CACHE[key] = _build(B, NCORES, GW)
    nc = _CACHE[key]

    in_maps, sq = _host_prepare(a, p, n, B, NCORES)
    res = run_bass_kernel_spmd(nc, in_maps, list(range(NCORES))).results
    outs = [res[k]["out"] for k in range(NCORES)]
    return _host_finalize(outs, sq, B, NCORES)


# revision 8
# speedup vs baseline: 1.0230x; 1.0230x over previous
"""BatchHardTripletLoss Trainium2 kernel (8 NeuronCores, SPMD).

Math: emb = concat(a,p,n) [3B, D]; labels tiled so same-label group of row r
is {a_i, p_i, n_i} with i = r mod B.  dist = sqrt(relu(d2)) is monotone in
d2 = sq_i + sq_j - 2*dot(e_i, e_j), so row max/min commute with sqrt:
  pos_max_d2[r] = max over the 2 partner rows (and 0 for self)
  neg_min_d2[r] = sq_r + min_j!=same (sq_j - 2 dot)
Each core handles 1536 rows x all 12288 cols. PE computes -2*dot into PSUM
(rows' lhsT pre-scaled by -2 on host); a tiny accumulate-matmul adds BIG on
the 3 same-label diagonals (made core-invariant by rotating each core's
column data by its row offset, host-side); one fused DVE tensor_tensor_reduce
per [128,2048] PSUM group adds the column sq and min-accumulates per row.
pos_max via small per-tile fused dot-reduces; regularizer via one abs-sum
reduce. Final sqrt/relu/means on host from tiny [128, 25] per-core outputs.
"""

import os
import sys

import numpy as np

try:
    import ml_dtypes

    BF16 = ml_dtypes.bfloat16
except ImportError:  # pragma: no cover
    BF16 = None

for _p in ("/opt/trn_rl_repo", os.path.expanduser("~/.axon_site/_ro/trn_rl_repo")):
    if os.path.isdir(_p) and _p not in sys.path:
        sys.path.insert(0, _p)
        break

B = 4096
D = 128
NCORES = 8
GW = 2048  # columns per PSUM group (4 banks)
MARGIN = 0.4
ALPHA = 0.01
BIG = float(2**20)

_CACHE = {}


def _build(b, ncores, gw):
    from contextlib import ExitStack

    import concourse.tile as tile
    from concourse import bacc, mybir

    tb = 3 * b
    rpc = tb // ncores  # rows per core
    rt_n = rpc // 128  # row tiles per core
    ng = tb // gw  # column groups
    nmm = gw // 512  # matmuls per group
    f32 = mybir.dt.float32
    bf16 = mybir.dt.bfloat16  # 1 cycle/row on PE, pipelined LDWEIGHTS
    Alu = mybir.AluOpType

    nc = bacc.Bacc("TRN2", target_bir_lowering=False, debug=False, num_devices=ncores)
    rhs_d = nc.dram_tensor("rhs", [128, tb], bf16, kind="ExternalInput")
    sqb_d = nc.dram_tensor("sqb", [128, tb], bf16, kind="ExternalInput")
    lhs_d = nc.dram_tensor("lhs", [128, rpc], bf16, kind="ExternalInput")
    identv_d = nc.dram_tensor("identv", [128, 128], bf16, kind="ExternalInput")
    bigI_d = nc.dram_tensor("bigI", [128, 128], bf16, kind="ExternalInput")
    e_d = nc.dram_tensor("erow", [rpc, 128], f32, kind="ExternalInput")
    o1_d = nc.dram_tensor("o1row", [rpc, 128], f32, kind="ExternalInput")
    o2_d = nc.dram_tensor("o2row", [rpc, 128], f32, kind="ExternalInput")
    s12_d = nc.dram_tensor("s12", [128, rt_n], f32, kind="ExternalInput")
    s13_d = nc.dram_tensor("s13", [128, rt_n], f32, kind="ExternalInput")
    out_d = nc.dram_tensor("out", [128, 3 * rt_n + 1], f32, kind="ExternalOutput")

    with tile.TileContext(nc) as tc, ExitStack() as ctx:
        singles = ctx.enter_context(tc.tile_pool(name="singles", bufs=1))
        psum_pool = ctx.enter_context(tc.tile_pool(name="psum", bufs=2, space="PSUM"))

        rhs_sb = singles.tile([128, tb], bf16)
        sqb_sb = singles.tile([128, tb], bf16)
        lhs_sb = singles.tile([128, rpc], bf16)
        for g in range(ng):
            sl = slice(g * gw, (g + 1) * gw)
            nc.sync.dma_start(out=rhs_sb[:, sl], in_=rhs_d[:, sl])
            nc.sync.dma_start(out=sqb_sb[:, sl], in_=sqb_d[:, sl])
        nc.sync.dma_start(out=lhs_sb[:], in_=lhs_d[:])

        e_sb = singles.tile([128, rt_n, 128], f32)
        o1_sb = singles.tile([128, rt_n, 128], f32)
        o2_sb = singles.tile([128, rt_n, 128], f32)
        nc.sync.dma_start(out=e_sb[:], in_=e_d[:].rearrange("(t p) d -> p t d", p=128))
        nc.sync.dma_start(
            out=o1_sb[:], in_=o1_d[:].rearrange("(t p) d -> p t d", p=128)
        )
        nc.sync.dma_start(
            out=o2_sb[:], in_=o2_d[:].rearrange("(t p) d -> p t d", p=128)
        )
        s12_sb = singles.tile([128, rt_n], f32)
        s13_sb = singles.tile([128, rt_n], f32)
        nc.sync.dma_start(out=s12_sb[:], in_=s12_d[:])
        nc.sync.dma_start(out=s13_sb[:], in_=s13_d[:])

        ident = singles.tile([128, 128], bf16)
        bigI = singles.tile([128, 128], bf16)
        nc.sync.dma_start(out=ident[:], in_=identv_d[:])
        nc.sync.dma_start(out=bigI[:], in_=bigI_d[:])

        negmin = singles.tile([128, rt_n], f32)
        negrm = singles.tile([128, rt_n], f32)
        stage_pool = ctx.enter_context(tc.tile_pool(name="stage", bufs=4))
        rm_pool = ctx.enter_context(tc.tile_pool(name="rm", bufs=2))
        posd1 = singles.tile([128, rt_n], f32)
        posd2 = singles.tile([128, rt_n], f32)
        posd = singles.tile([128, rt_n], f32)
        abss = singles.tile([128, 1], f32)

        rhs_r = rhs_sb[:]
        sqb_r = sqb_sb[:]
        lhs_r = lhs_sb[:]
        ident_r = ident[:]
        bigI_r = bigI[:]

        for rt in range(rt_n):
            lhs_slice = lhs_r[:, rt * 128 : (rt + 1) * 128]
            # same-label diagonals sit at local column t*b + rt*128 (core
            # invariant thanks to the host-side column rotation)
            mask_cols = [t * b + rt * 128 for t in range(3)]
            for g in range(ng):
                ps = psum_pool.tile([128, gw], f32)
                cm = None
                for c in mask_cols:
                    if g * gw <= c < (g + 1) * gw:
                        cm = c - g * gw
                for s in range(nmm):
                    # -2*dot
                    nc.tensor.matmul(
                        ps[:, s * 512 : (s + 1) * 512],
                        lhs_slice,
                        rhs_r[:, g * gw + s * 512 : g * gw + (s + 1) * 512],
                        start=True,
                        stop=False,
                    )
                for s in range(nmm):
                    # += sq[j]  (identity-weighted copy of the sq row-bcast)
                    masked_here = cm is not None and s * 512 <= cm < (s + 1) * 512
                    nc.tensor.matmul(
                        ps[:, s * 512 : (s + 1) * 512],
                        ident_r,
                        sqb_r[:, g * gw + s * 512 : g * gw + (s + 1) * 512],
                        start=False,
                        stop=not masked_here,
                    )
                if cm is not None:
                    # += BIG on the same-label diagonal
                    nc.tensor.matmul(
                        ps[:, cm : cm + 128],
                        ident_r,
                        bigI_r,
                        start=False,
                        stop=True,
                    )
                if g == 0:
                    # direct path: plain PSUM row-min on DVE
                    nc.vector.tensor_reduce(
                        out=negmin[:, rt : rt + 1],
                        in_=ps[:],
                        axis=mybir.AxisListType.X,
                        op=Alu.min,
                    )
                else:
                    # staged path: ACT evicts+casts PSUM -> bf16 SBUF, DVE
                    # folds into the per-rt running elementwise min at 2x
                    st = stage_pool.tile([128, gw], bf16)
                    nc.scalar.activation(
                        out=st[:],
                        in_=ps[:],
                        func=mybir.ActivationFunctionType.Copy,
                    )
                    if g == 1:
                        st_first = st
                    elif g == 2:
                        rm = rm_pool.tile([128, gw], bf16)
                        nc.vector.tensor_tensor(
                            out=rm[:], in0=st_first[:], in1=st[:], op=Alu.min
                        )
                    else:
                        nc.vector.tensor_tensor(
                            out=rm[:], in0=rm[:], in1=st[:], op=Alu.min
                        )
            nc.vector.tensor_reduce(
                out=negrm[:, rt : rt + 1],
                in_=rm[:],
                axis=mybir.AxisListType.X,
                op=Alu.min,
            )

        # pos path: d2(e,o) = (sq_e + sq_o) - 2*sum_d(e*o), batched over tiles
        prod = singles.tile([128, rt_n, 128], f32)
        dots1 = singles.tile([128, rt_n], f32)
        dots2 = singles.tile([128, rt_n], f32)
        prod2 = singles.tile([128, rt_n, 128], f32)
        nc.gpsimd.tensor_mul(prod[:], e_sb[:], o1_sb[:])
        nc.vector.tensor_reduce(
            out=dots1[:], in_=prod[:], axis=mybir.AxisListType.X, op=Alu.add
        )
        nc.gpsimd.tensor_mul(prod2[:], e_sb[:], o2_sb[:])
        nc.vector.tensor_reduce(
            out=dots2[:], in_=prod2[:], axis=mybir.AxisListType.X, op=Alu.add
        )
        nc.vector.tensor_scalar(
            out=dots1[:], in0=dots1[:], scalar1=-2.0, scalar2=None, op0=Alu.mult
        )
        nc.vector.tensor_scalar(
            out=dots2[:], in0=dots2[:], scalar1=-2.0, scalar2=None, op0=Alu.mult
        )
        nc.vector.tensor_add(out=posd1[:], in0=dots1[:], in1=s12_sb[:])
        nc.vector.tensor_add(out=posd2[:], in0=dots2[:], in1=s13_sb[:])
        nc.vector.tensor_max(out=posd[:], in0=posd1[:], in1=posd2[:])

        nc.vector.tensor_reduce(
            out=abss[:],
            in_=rhs_sb[:, 0:rpc],
            axis=mybir.AxisListType.X,
            op=Alu.add,
            apply_absolute_value=True,
        )

        nc.sync.dma_start(out=out_d[:, 0:rt_n], in_=negmin[:])
        nc.sync.dma_start(out=out_d[:, rt_n : 2 * rt_n], in_=posd[:])
        nc.sync.dma_start(out=out_d[:, 2 * rt_n : 3 * rt_n], in_=negrm[:])
        nc.sync.dma_start(out=out_d[:, 3 * rt_n : 3 * rt_n + 1], in_=abss[:])

    nc.compile()
    return nc


def _host_prepare(a, p, n, b, ncores):
    tb = 3 * b
    rpc = tb // ncores
    rt_n = rpc // 128
    emb = np.concatenate([a, p, n], axis=0).astype(np.float32)  # [3b, D]
    embT = np.ascontiguousarray(emb.T)  # [D, 3b]
    sq = (emb * emb).sum(axis=1, dtype=np.float32)  # [3b]

    in_maps = []
    for k in range(ncores):
        r0 = k * rpc
        rot = (np.arange(tb) + r0) % tb  # local col j holds global col j+r0
        rhs_k = np.ascontiguousarray(embT[:, rot]).astype(BF16)
        sqb_k = np.ascontiguousarray(
            np.broadcast_to(sq[rot][None, :], (128, tb))
        ).astype(BF16)
        lhs_k = np.ascontiguousarray(-2.0 * embT[:, r0 : r0 + rpc]).astype(BF16)
        idx = np.arange(r0, r0 + rpc)
        i = idx % b
        w = idx // b
        o1_idx = ((w + 1) % 3) * b + i
        o2_idx = ((w + 2) % 3) * b + i
        s12_k = np.ascontiguousarray((sq[idx] + sq[o1_idx]).reshape(rt_n, 128).T)
        s13_k = np.ascontiguousarray((sq[idx] + sq[o2_idx]).reshape(rt_n, 128).T)
        in_maps.append(
            {
                "rhs": rhs_k,
                "sqb": sqb_k,
                "lhs": lhs_k,
                "erow": np.ascontiguousarray(emb[idx]),
                "o1row": np.ascontiguousarray(emb[o1_idx]),
                "o2row": np.ascontiguousarray(emb[o2_idx]),
                "s12": s12_k,
                "s13": s13_k,
                "identv": np.eye(128, dtype=np.float32).astype(BF16),
                "bigI": (np.eye(128, dtype=np.float32) * np.float32(BIG)).astype(BF16),
            }
        )
    return in_maps, sq


def _host_finalize(outs, sq, b, ncores):
    """outs: list (per core) of [128, 2*rt_n+1] fp32 device outputs."""
    tb = 3 * b
    rpc = tb // ncores
    rt_n = rpc // 128
    loss_sum = 0.0
    abs_sum = 0.0
    for k in range(ncores):
        o = np.asarray(outs[k], dtype=np.float64)
        idx = np.arange(k * rpc, (k + 1) * rpc)
        sq_r = sq[idx].reshape(rt_n, 128).T  # [p, rt]
        neg_d2 = np.minimum(o[:, 0:rt_n], o[:, 2 * rt_n : 3 * rt_n]) + sq_r
        pos_d2 = o[:, rt_n : 2 * rt_n]
        neg = np.sqrt(np.maximum(neg_d2, 0.0))
        pos = np.sqrt(np.maximum(pos_d2, 0.0))
        loss_sum += np.maximum(pos - neg + MARGIN, 0.0).sum()
        abs_sum += o[:, 3 * rt_n].sum()
    loss = loss_sum / tb
    sq_sum = sq.sum(dtype=np.float64)
    cnt = tb * D
    reg = (sq_sum - 2.0 * abs_sum + cnt) / cnt
    return np.float32(loss + ALPHA * reg)


def kernel(a, p, n):
    from concourse.bass_utils import run_bass_kernel_spmd

    a = np.asarray(a, dtype=np.float32)
    p = np.asarray(p, dtype=np.float32)
    n = np.asarray(n, dtype=np.float32)
    assert a.shape == (B, D) and p.shape == (B, D) and n.shape == (B, D)

    key = (B, NCORES, GW)
    if key not in _CACHE:
        _CACHE[key] = _build(B, NCORES, GW)
    nc = _CACHE[key]

    in_maps, sq = _host_prepare(a, p, n, B, NCORES)
    res = run_bass_kernel_spmd(nc, in_maps, list(range(NCORES))).results
    outs = [res[k]["out"] for k in range(NCORES)]
    return _host_finalize(outs, sq, B, NCORES)


# revision 12
# speedup vs baseline: 1.2428x; 1.2149x over previous
"""BatchHardTripletLoss Trainium2 kernel (8 NeuronCores, SPMD).

Math: emb = concat(a,p,n) [3B, D]; labels tiled so same-label group of row r
is {a_i, p_i, n_i} with i = r mod B.  dist = sqrt(relu(d2)) is monotone in
d2 = sq_i + sq_j - 2*dot(e_i, e_j), so row max/min commute with sqrt:
  pos_max_d2[r] = max over the 2 partner rows (and 0 for self)
  neg_min_d2[r] = sq_r + min_j!=same (sq_j - 2 dot)
Each core handles 1536 rows x all 12288 cols. PE computes -2*dot into PSUM
(rows' lhsT pre-scaled by -2 on host); a tiny accumulate-matmul adds BIG on
the 3 same-label diagonals (made core-invariant by rotating each core's
column data by its row offset, host-side); one fused DVE tensor_tensor_reduce
per [128,2048] PSUM group adds the column sq and min-accumulates per row.
pos_max via small per-tile fused dot-reduces; regularizer via one abs-sum
reduce. Final sqrt/relu/means on host from tiny [128, 25] per-core outputs.
"""

import os
import sys

import numpy as np

try:
    import ml_dtypes

    BF16 = ml_dtypes.bfloat16
except ImportError:  # pragma: no cover
    BF16 = None

for _p in ("/opt/trn_rl_repo", os.path.expanduser("~/.axon_site/_ro/trn_rl_repo")):
    if os.path.isdir(_p) and _p not in sys.path:
        sys.path.insert(0, _p)
        break

B = 4096
D = 128
NCORES = 8
GW = 2048  # columns per PSUM group (4 banks)
MARGIN = 0.4
ALPHA = 0.01
BIG = float(2**20)

_CACHE = {}


def _build(b, ncores, gw=None):
    from contextlib import ExitStack

    import concourse.tile as tile
    from concourse import bacc, mybir

    tb = 3 * b
    rpc = tb // ncores  # rows per core
    rt_n = rpc // 128  # row tiles per core
    nct = tb // 128  # column tiles (128 stationary columns each)
    bt = b // 128  # column tiles per b-block
    mw = 512  # moving width per matmul (one PSUM bank)
    nmv = (rpc + mw - 1) // mw  # matmuls per column tile
    f32 = mybir.dt.float32
    bf16 = mybir.dt.bfloat16
    Alu = mybir.AluOpType
    AF = mybir.ActivationFunctionType

    # fraction of column tiles evicted by ACT (rest by DVE tensor_scalar)
    act_tiles = [c for c in range(nct) if (c * 4) // 5 != ((c + 1) * 4) // 5]

    nc = bacc.Bacc("TRN2", target_bir_lowering=False, debug=False, num_devices=ncores)
    rhs_d = nc.dram_tensor("rhs", [128, tb], bf16, kind="ExternalInput")
    lhs_d = nc.dram_tensor("lhs", [128, rpc], bf16, kind="ExternalInput")
    sqt_d = nc.dram_tensor("sqt", [128, nct], f32, kind="ExternalInput")
    identv_d = nc.dram_tensor("identv", [128, 128], bf16, kind="ExternalInput")
    bigI_d = nc.dram_tensor("bigI", [128, 128], bf16, kind="ExternalInput")
    e_d = nc.dram_tensor("erow", [rpc, 128], f32, kind="ExternalInput")
    o1_d = nc.dram_tensor("o1row", [rpc, 128], f32, kind="ExternalInput")
    o2_d = nc.dram_tensor("o2row", [rpc, 128], f32, kind="ExternalInput")
    s12_d = nc.dram_tensor("s12", [128, rt_n], f32, kind="ExternalInput")
    s13_d = nc.dram_tensor("s13", [128, rt_n], f32, kind="ExternalInput")
    outrm_d = nc.dram_tensor("outrm", [128, rpc], bf16, kind="ExternalOutput")
    out_d = nc.dram_tensor("out", [128, rt_n + 1], f32, kind="ExternalOutput")

    with tile.TileContext(nc) as tc, ExitStack() as ctx:
        singles = ctx.enter_context(tc.tile_pool(name="singles", bufs=1))
        stage_pool = ctx.enter_context(tc.tile_pool(name="stage", bufs=4))
        psum_pool = ctx.enter_context(tc.tile_pool(name="psum", bufs=2, space="PSUM"))

        rhs_sb = singles.tile([128, tb], bf16)
        lhs_sb = singles.tile([128, rpc], bf16)
        sqt_sb = singles.tile([128, nct], f32)
        ident = singles.tile([128, 128], bf16)
        bigI = singles.tile([128, 128], bf16)
        # load order: first compute tile's deps first
        nc.sync.dma_start(out=lhs_sb[:], in_=lhs_d[:])
        nc.sync.dma_start(out=sqt_sb[:], in_=sqt_d[:])
        nc.sync.dma_start(out=ident[:], in_=identv_d[:])
        nc.sync.dma_start(out=bigI[:], in_=bigI_d[:])
        csz = tb // 6
        for g in range(6):
            sl = slice(g * csz, (g + 1) * csz)
            nc.sync.dma_start(out=rhs_sb[:, sl], in_=rhs_d[:, sl])

        e_sb = singles.tile([128, rt_n, 128], f32)
        o1_sb = singles.tile([128, rt_n, 128], f32)
        o2_sb = singles.tile([128, rt_n, 128], f32)
        nc.sync.dma_start(out=e_sb[:], in_=e_d[:].rearrange("(t p) d -> p t d", p=128))
        nc.sync.dma_start(
            out=o1_sb[:], in_=o1_d[:].rearrange("(t p) d -> p t d", p=128)
        )
        nc.sync.dma_start(
            out=o2_sb[:], in_=o2_d[:].rearrange("(t p) d -> p t d", p=128)
        )
        s12_sb = singles.tile([128, rt_n], f32)
        s13_sb = singles.tile([128, rt_n], f32)
        nc.sync.dma_start(out=s12_sb[:], in_=s12_d[:])
        nc.sync.dma_start(out=s13_sb[:], in_=s13_d[:])

        posd1 = singles.tile([128, rt_n], f32)
        posd2 = singles.tile([128, rt_n], f32)
        posd = singles.tile([128, rt_n], f32)
        abss = singles.tile([128, 1], f32)
        rm = singles.tile([128, rpc], bf16)

        # pos path: d2(e,o) = (sq_e + sq_o) - 2*sum_d(e*o)
        prod = singles.tile([128, rt_n, 128], f32)
        prod2 = singles.tile([128, rt_n, 128], f32)
        dots1 = singles.tile([128, rt_n], f32)
        dots2 = singles.tile([128, rt_n], f32)
        nc.gpsimd.tensor_mul(prod[:], e_sb[:], o1_sb[:])
        nc.vector.tensor_reduce(
            out=dots1[:], in_=prod[:], axis=mybir.AxisListType.X, op=Alu.add
        )
        nc.gpsimd.tensor_mul(prod2[:], e_sb[:], o2_sb[:])
        nc.vector.tensor_reduce(
            out=dots2[:], in_=prod2[:], axis=mybir.AxisListType.X, op=Alu.add
        )
        nc.vector.tensor_scalar(
            out=dots1[:], in0=dots1[:], scalar1=-2.0, scalar2=None, op0=Alu.mult
        )
        nc.vector.tensor_scalar(
            out=dots2[:], in0=dots2[:], scalar1=-2.0, scalar2=None, op0=Alu.mult
        )
        nc.vector.tensor_add(out=posd1[:], in0=dots1[:], in1=s12_sb[:])
        nc.vector.tensor_add(out=posd2[:], in0=dots2[:], in1=s13_sb[:])
        nc.vector.tensor_max(out=posd[:], in0=posd1[:], in1=posd2[:])

        nc.vector.tensor_reduce(
            out=abss[:],
            in_=rhs_sb[:, 0:rpc],
            axis=mybir.AxisListType.X,
            op=Alu.add,
            apply_absolute_value=True,
        )

        st_first = None
        for c in range(nct):
            # psum tile: 128 stationary columns x all of this core's rows
            ps = psum_pool.tile([128, rpc], f32)
            q = c % bt
            mask_bank = (q * 128) // mw if q < rt_n else None
            for s in range(nmv):
                n0 = s * mw
                n1 = min(rpc, n0 + mw)
                nc.tensor.matmul(
                    ps[:, n0:n1],
                    rhs_sb[:, c * 128 : (c + 1) * 128],
                    lhs_sb[:, n0:n1],
                    start=True,
                    stop=s != mask_bank,
                )
            if mask_bank is not None:
                # same-label diagonal: (p, i=q*128+p) += BIG
                nc.tensor.matmul(
                    ps[:, q * 128 : q * 128 + 128],
                    ident[:],
                    bigI[:],
                    start=False,
                    stop=True,
                )
            st = stage_pool.tile([128, rpc], bf16)
            if c in act_tiles:
                nc.scalar.activation(
                    out=st[:],
                    in_=ps[:],
                    func=AF.Identity,
                    bias=sqt_sb[:, c : c + 1],
                )
            else:
                nc.vector.tensor_scalar(
                    out=st[:],
                    in0=ps[:],
                    scalar1=sqt_sb[:, c : c + 1],
                    scalar2=None,
                    op0=Alu.add,
                )
            if c == 0:
                st_first = st
            elif c == 1:
                nc.vector.tensor_tensor(
                    out=rm[:], in0=st_first[:], in1=st[:], op=Alu.min
                )
            else:
                nc.vector.tensor_tensor(out=rm[:], in0=rm[:], in1=st[:], op=Alu.min)

        nc.sync.dma_start(out=outrm_d[:], in_=rm[:])
        nc.sync.dma_start(out=out_d[:, 0:rt_n], in_=posd[:])
        nc.sync.dma_start(out=out_d[:, rt_n : rt_n + 1], in_=abss[:])

    nc.compile()
    return nc


def _host_prepare(a, p, n, b, ncores):
    tb = 3 * b
    rpc = tb // ncores
    rt_n = rpc // 128
    emb = np.concatenate([a, p, n], axis=0).astype(np.float32)  # [3b, D]
    embT = np.ascontiguousarray(emb.T)  # [D, 3b]
    sq = (emb * emb).sum(axis=1, dtype=np.float32)  # [3b]

    in_maps = []
    for k in range(ncores):
        r0 = k * rpc
        rot = (np.arange(tb) + r0) % tb  # local col j holds global col j+r0
        rhs_k = np.ascontiguousarray(embT[:, rot]).astype(BF16)
        sqt_k = np.ascontiguousarray(sq[rot].reshape(tb // 128, 128).T)
        lhs_k = np.ascontiguousarray(-2.0 * embT[:, r0 : r0 + rpc]).astype(BF16)
        idx = np.arange(r0, r0 + rpc)
        i = idx % b
        w = idx // b
        o1_idx = ((w + 1) % 3) * b + i
        o2_idx = ((w + 2) % 3) * b + i
        s12_k = np.ascontiguousarray((sq[idx] + sq[o1_idx]).reshape(rt_n, 128).T)
        s13_k = np.ascontiguousarray((sq[idx] + sq[o2_idx]).reshape(rt_n, 128).T)
        in_maps.append(
            {
                "rhs": rhs_k,
                "sqt": sqt_k,
                "lhs": lhs_k,
                "erow": np.ascontiguousarray(emb[idx]),
                "o1row": np.ascontiguousarray(emb[o1_idx]),
                "o2row": np.ascontiguousarray(emb[o2_idx]),
                "s12": s12_k,
                "s13": s13_k,
                "identv": np.eye(128, dtype=np.float32).astype(BF16),
                "bigI": (np.eye(128, dtype=np.float32) * np.float32(BIG)).astype(BF16),
            }
        )
    return in_maps, sq


def _host_finalize(outs, sq, b, ncores):
    """outs: list (per core) of (rm [128, rpc] bf16, small [128, rt_n+1] f32)."""
    tb = 3 * b
    rpc = tb // ncores
    rt_n = rpc // 128
    loss_sum = 0.0
    abs_sum = 0.0
    for k in range(ncores):
        rm, o = outs[k]
        rm = np.asarray(rm, dtype=np.float64)  # [128, rpc]
        o = np.asarray(o, dtype=np.float64)
        idx = np.arange(k * rpc, (k + 1) * rpc)
        neg_d2 = rm.min(axis=0) + sq[idx]  # [rpc]
        pos_d2 = o[:, 0:rt_n].T.reshape(rpc)  # [rt, p] -> row t*128+p
        neg = np.sqrt(np.maximum(neg_d2, 0.0))
        pos = np.sqrt(np.maximum(pos_d2, 0.0))
        loss_sum += np.maximum(pos - neg + MARGIN, 0.0).sum()
        abs_sum += o[:, rt_n].sum()
    loss = loss_sum / tb
    sq_sum = sq.sum(dtype=np.float64)
    cnt = tb * D
    reg = (sq_sum - 2.0 * abs_sum + cnt) / cnt
    return np.float32(loss + ALPHA * reg)


def kernel(a, p, n):
    from concourse.bass_utils import run_bass_kernel_spmd

    a = np.asarray(a, dtype=np.float32)
    p = np.asarray(p, dtype=np.float32)
    n = np.asarray(n, dtype=np.float32)
    assert a.shape == (B, D) and p.shape == (B, D) and n.shape == (B, D)

    key = (B, NCORES, GW)
    if key not in _CACHE:
        _CACHE[key] = _build(B, NCORES, GW)
    nc = _CACHE[key]

    in_maps, sq = _host_prepare(a, p, n, B, NCORES)
    res = run_bass_kernel_spmd(nc, in_maps, list(range(NCORES))).results
    outs = [(res[k]["outrm"], res[k]["out"]) for k in range(NCORES)]
    return _host_finalize(outs, sq, B, NCORES)


# revision 13
# speedup vs baseline: 1.2599x; 1.0137x over previous
"""BatchHardTripletLoss Trainium2 kernel (8 NeuronCores, SPMD).

Math: emb = concat(a,p,n) [3B, D]; labels tiled so same-label group of row r
is {a_i, p_i, n_i} with i = r mod B.  dist = sqrt(relu(d2)) is monotone in
d2 = sq_i + sq_j - 2*dot(e_i, e_j), so row max/min commute with sqrt:
  pos_max_d2[r] = max over the 2 partner rows (and 0 for self)
  neg_min_d2[r] = sq_r + min_j!=same (sq_j - 2 dot)
Each core handles 1536 rows x all 12288 cols. PE computes -2*dot into PSUM
(rows' lhsT pre-scaled by -2 on host); a tiny accumulate-matmul adds BIG on
the 3 same-label diagonals (made core-invariant by rotating each core's
column data by its row offset, host-side); one fused DVE tensor_tensor_reduce
per [128,2048] PSUM group adds the column sq and min-accumulates per row.
pos_max via small per-tile fused dot-reduces; regularizer via one abs-sum
reduce. Final sqrt/relu/means on host from tiny [128, 25] per-core outputs.
"""

import os
import sys

import numpy as np

try:
    import ml_dtypes

    BF16 = ml_dtypes.bfloat16
except ImportError:  # pragma: no cover
    BF16 = None

for _p in ("/opt/trn_rl_repo", os.path.expanduser("~/.axon_site/_ro/trn_rl_repo")):
    if os.path.isdir(_p) and _p not in sys.path:
        sys.path.insert(0, _p)
        break

B = 4096
D = 128
NCORES = 8
GW = 2048  # columns per PSUM group (4 banks)
MARGIN = 0.4
ALPHA = 0.01
BIG = float(2**20)

_CACHE = {}


def _build(b, ncores, gw=None):
    from contextlib import ExitStack

    import concourse.tile as tile
    from concourse import bacc, mybir

    tb = 3 * b
    rpc = tb // ncores  # rows per core
    rt_n = rpc // 128  # row tiles per core
    nct = tb // 128  # column tiles (128 stationary columns each)
    bt = b // 128  # column tiles per b-block
    mw = 512  # moving width per matmul (one PSUM bank)
    nmv = (rpc + mw - 1) // mw  # matmuls per column tile
    f32 = mybir.dt.float32
    bf16 = mybir.dt.bfloat16
    Alu = mybir.AluOpType
    AF = mybir.ActivationFunctionType

    # fraction of column tiles evicted by ACT (rest by DVE tensor_scalar)
    act_tiles = [c for c in range(nct) if c % 7 != 3]

    nc = bacc.Bacc("TRN2", target_bir_lowering=False, debug=False, num_devices=ncores)
    rhs_d = nc.dram_tensor("rhs", [128, tb], bf16, kind="ExternalInput")
    lhs_d = nc.dram_tensor("lhs", [128, rpc], bf16, kind="ExternalInput")
    sqt_d = nc.dram_tensor("sqt", [128, nct], f32, kind="ExternalInput")
    identv_d = nc.dram_tensor("identv", [128, 128], bf16, kind="ExternalInput")
    bigI_d = nc.dram_tensor("bigI", [128, 128], bf16, kind="ExternalInput")
    e_d = nc.dram_tensor("erow", [rpc, 128], f32, kind="ExternalInput")
    o1_d = nc.dram_tensor("o1row", [rpc, 128], f32, kind="ExternalInput")
    o2_d = nc.dram_tensor("o2row", [rpc, 128], f32, kind="ExternalInput")
    s12_d = nc.dram_tensor("s12", [128, rt_n], f32, kind="ExternalInput")
    s13_d = nc.dram_tensor("s13", [128, rt_n], f32, kind="ExternalInput")
    outrm_d = nc.dram_tensor("outrm", [128, rpc], bf16, kind="ExternalOutput")
    out_d = nc.dram_tensor("out", [128, rt_n + 1], f32, kind="ExternalOutput")

    with tile.TileContext(nc) as tc, ExitStack() as ctx:
        singles = ctx.enter_context(tc.tile_pool(name="singles", bufs=1))
        stage_pool = ctx.enter_context(tc.tile_pool(name="stage", bufs=4))
        psum_pool = ctx.enter_context(tc.tile_pool(name="psum", bufs=2, space="PSUM"))

        rhs_sb = singles.tile([128, tb], bf16)
        lhs_sb = singles.tile([128, rpc], bf16)
        sqt_sb = singles.tile([128, nct], f32)
        ident = singles.tile([128, 128], bf16)
        bigI = singles.tile([128, 128], bf16)
        # load order: first compute tile's deps first
        nc.sync.dma_start(out=lhs_sb[:], in_=lhs_d[:])
        nc.sync.dma_start(out=sqt_sb[:], in_=sqt_d[:])
        nc.sync.dma_start(out=ident[:], in_=identv_d[:])
        nc.sync.dma_start(out=bigI[:], in_=bigI_d[:])
        csz = tb // 6
        for g in range(6):
            sl = slice(g * csz, (g + 1) * csz)
            nc.sync.dma_start(out=rhs_sb[:, sl], in_=rhs_d[:, sl])

        e_sb = singles.tile([128, rt_n, 128], f32)
        o1_sb = singles.tile([128, rt_n, 128], f32)
        o2_sb = singles.tile([128, rt_n, 128], f32)
        nc.sync.dma_start(out=e_sb[:], in_=e_d[:].rearrange("(t p) d -> p t d", p=128))
        nc.sync.dma_start(
            out=o1_sb[:], in_=o1_d[:].rearrange("(t p) d -> p t d", p=128)
        )
        nc.sync.dma_start(
            out=o2_sb[:], in_=o2_d[:].rearrange("(t p) d -> p t d", p=128)
        )
        s12_sb = singles.tile([128, rt_n], f32)
        s13_sb = singles.tile([128, rt_n], f32)
        nc.sync.dma_start(out=s12_sb[:], in_=s12_d[:])
        nc.sync.dma_start(out=s13_sb[:], in_=s13_d[:])

        posd1 = singles.tile([128, rt_n], f32)
        posd2 = singles.tile([128, rt_n], f32)
        posd = singles.tile([128, rt_n], f32)
        abss = singles.tile([128, 1], f32)
        rm = singles.tile([128, rpc], bf16)

        # pos path: d2(e,o) = (sq_e + sq_o) - 2*sum_d(e*o)
        prod = singles.tile([128, rt_n, 128], f32)
        prod2 = singles.tile([128, rt_n, 128], f32)
        dots1 = singles.tile([128, rt_n], f32)
        dots2 = singles.tile([128, rt_n], f32)
        nc.gpsimd.tensor_mul(prod[:], e_sb[:], o1_sb[:])
        nc.vector.tensor_reduce(
            out=dots1[:], in_=prod[:], axis=mybir.AxisListType.X, op=Alu.add
        )
        nc.gpsimd.tensor_mul(prod2[:], e_sb[:], o2_sb[:])
        nc.vector.tensor_reduce(
            out=dots2[:], in_=prod2[:], axis=mybir.AxisListType.X, op=Alu.add
        )
        nc.vector.tensor_scalar(
            out=dots1[:], in0=dots1[:], scalar1=-2.0, scalar2=None, op0=Alu.mult
        )
        nc.vector.tensor_scalar(
            out=dots2[:], in0=dots2[:], scalar1=-2.0, scalar2=None, op0=Alu.mult
        )
        nc.vector.tensor_add(out=posd1[:], in0=dots1[:], in1=s12_sb[:])
        nc.vector.tensor_add(out=posd2[:], in0=dots2[:], in1=s13_sb[:])
        nc.vector.tensor_max(out=posd[:], in0=posd1[:], in1=posd2[:])

        nc.vector.tensor_reduce(
            out=abss[:],
            in_=rhs_sb[:, 0:rpc],
            axis=mybir.AxisListType.X,
            op=Alu.add,
            apply_absolute_value=True,
        )

        st_first = None
        for c in range(nct):
            # psum tile: 128 stationary columns x all of this core's rows
            ps = psum_pool.tile([128, rpc], f32)
            q = c % bt
            mask_bank = (q * 128) // mw if q < rt_n else None
            for s in range(nmv):
                n0 = s * mw
                n1 = min(rpc, n0 + mw)
                nc.tensor.matmul(
                    ps[:, n0:n1],
                    rhs_sb[:, c * 128 : (c + 1) * 128],
                    lhs_sb[:, n0:n1],
                    start=True,
                    stop=s != mask_bank,
                )
            if mask_bank is not None:
                # same-label diagonal: (p, i=q*128+p) += BIG
                nc.tensor.matmul(
                    ps[:, q * 128 : q * 128 + 128],
                    ident[:],
                    bigI[:],
                    start=False,
                    stop=True,
                )
            st = stage_pool.tile([128, rpc], bf16)
            if c in act_tiles:
                nc.scalar.activation(
                    out=st[:],
                    in_=ps[:],
                    func=AF.Identity,
                    bias=sqt_sb[:, c : c + 1],
                )
            else:
                nc.vector.tensor_scalar(
                    out=st[:],
                    in0=ps[:],
                    scalar1=sqt_sb[:, c : c + 1],
                    scalar2=None,
                    op0=Alu.add,
                )
            if c == 0:
                st_first = st
            elif c == 1:
                nc.vector.tensor_tensor(
                    out=rm[:], in0=st_first[:], in1=st[:], op=Alu.min
                )
            else:
                nc.vector.tensor_tensor(out=rm[:], in0=rm[:], in1=st[:], op=Alu.min)

        nc.sync.dma_start(out=outrm_d[:], in_=rm[:])
        nc.sync.dma_start(out=out_d[:, 0:rt_n], in_=posd[:])
        nc.sync.dma_start(out=out_d[:, rt_n : rt_n + 1], in_=abss[:])

    nc.compile()
    return nc


def _host_prepare(a, p, n, b, ncores):
    tb = 3 * b
    rpc = tb // ncores
    rt_n = rpc // 128
    emb = np.concatenate([a, p, n], axis=0).astype(np.float32)  # [3b, D]
    embT = np.ascontiguousarray(emb.T)  # [D, 3b]
    sq = (emb * emb).sum(axis=1, dtype=np.float32)  # [3b]

    in_maps = []
    for k in range(ncores):
        r0 = k * rpc
        rot = (np.arange(tb) + r0) % tb  # local col j holds global col j+r0
        rhs_k = np.ascontiguousarray(embT[:, rot]).astype(BF16)
        sqt_k = np.ascontiguousarray(sq[rot].reshape(tb // 128, 128).T)
        lhs_k = np.ascontiguousarray(-2.0 * embT[:, r0 : r0 + rpc]).astype(BF16)
        idx = np.arange(r0, r0 + rpc)
        i = idx % b
        w = idx // b
        o1_idx = ((w + 1) % 3) * b + i
        o2_idx = ((w + 2) % 3) * b + i
        s12_k = np.ascontiguousarray((sq[idx] + sq[o1_idx]).reshape(rt_n, 128).T)
        s13_k = np.ascontiguousarray((sq[idx] + sq[o2_idx]).reshape(rt_n, 128).T)
        in_maps.append(
            {
                "rhs": rhs_k,
                "sqt": sqt_k,
                "lhs": lhs_k,
                "erow": np.ascontiguousarray(emb[idx]),
                "o1row": np.ascontiguousarray(emb[o1_idx]),
                "o2row": np.ascontiguousarray(emb[o2_idx]),
                "s12": s12_k,
                "s13": s13_k,
                "identv": np.eye(128, dtype=np.float32).astype(BF16),
                "bigI": (np.eye(128, dtype=np.float32) * np.float32(BIG)).astype(BF16),
            }
        )
    return in_maps, sq


def _host_finalize(outs, sq, b, ncores):
    """outs: list (per core) of (rm [128, rpc] bf16, small [128, rt_n+1] f32)."""
    tb = 3 * b
    rpc = tb // ncores
    rt_n = rpc // 128
    loss_sum = 0.0
    abs_sum = 0.0
    for k in range(ncores):
        rm, o = outs[k]
        rm = np.asarray(rm, dtype=np.float64)  # [128, rpc]
        o = np.asarray(o, dtype=np.float64)
        idx = np.arange(k * rpc, (k + 1) * rpc)
        neg_d2 = rm.min(axis=0) + sq[idx]  # [rpc]
        pos_d2 = o[:, 0:rt_n].T.reshape(rpc)  # [rt, p] -> row t*128+p
        neg = np.sqrt(np.maximum(neg_d2, 0.0))
        pos = np.sqrt(np.maximum(pos_d2, 0.0))
        loss_sum += np.maximum(pos - neg + MARGIN, 0.0).sum()
        abs_sum += o[:, rt_n].sum()
    loss = loss_sum / tb
    sq_sum = sq.sum(dtype=np.float64)
    cnt = tb * D
    reg = (sq_sum - 2.0 * abs_sum + cnt) / cnt
    return np.float32(loss + ALPHA * reg)


def kernel(a, p, n):
    from concourse.bass_utils import run_bass_kernel_spmd

    a = np.asarray(a, dtype=np.float32)
    p = np.asarray(p, dtype=np.float32)
    n = np.asarray(n, dtype=np.float32)
    assert a.shape == (B, D) and p.shape == (B, D) and n.shape == (B, D)

    key = (B, NCORES, GW)
    if key not in _CACHE:
        _CACHE[key] = _build(B, NCORES, GW)
    nc = _CACHE[key]

    in_maps, sq = _host_prepare(a, p, n, B, NCORES)
    res = run_bass_kernel_spmd(nc, in_maps, list(range(NCORES))).results
    outs = [(res[k]["outrm"], res[k]["out"]) for k in range(NCORES)]
    return _host_finalize(outs, sq, B, NCORES)
